# revision 45
# baseline (speedup 1.0000x reference)
"""Distributed Trainium2 Bass kernel for a post-LN transformer layer (v5).

Problem nn_AttentionLayer_257698038341:
    x: (L=2048, B=4, D=1024), H=16 heads, DFF=4096, fp32, exact GELU.

Sharding (zero collectives): core i owns batch g=i//2 and half hf=i%2 of
the sequence (1024 contiguous tokens after a host-side roll). Each core
computes K/V for its batch's full 2048 tokens (the only duplicated
work), and Q / attention / O-proj / LN1 / FFN / LN2 for its own 1024.

Perf structure (v5):
  * fp8e4 + DoubleRow matmuls for Q/K/V projections, attn@V, and the
    O-projection. These feed attn_out, which is only ~4% of the
    residual magnitude, so fp8 quantization error is diluted ~30x.
    Weights pre-scaled x8 on the host; descales folded into the exp
    scale (1/64) and the O-proj residual add (1/64). FFN stays bf16:
    its output is ~50% of the final variance.
  * softmax exp computes exp(s - 3) (shift-invariant) so fp8
    probabilities stay below fp8e4's 240 max.
  * head-PAIR packed scores: the two heads of chunk m live at
    partitions 0-63 / 64-127, so their 64-row matmuls target different
    PE row-groups and run concurrently; exp is issued over 4 PSUM
    banks (2 kt x 2 heads = 2048 elems) to amortize ACT overheads.
  * attention is ACT(exp)-bound, so independent matmul work is
    interleaved into each head-pair's instruction stream to keep PE
    busy between exp-gated segments: qb0's attention absorbs the K/Q
    projections; qb1's attention absorbs qb0's FFN1. FFN1 drains PSUM
    via DVE (bias add, raw z) and one giant in-place GELU per qb runs
    after the exp batch, avoiding ACT table thrash.
  * LN rstd = exp(-0.5*ln(var+eps)) keeps the LN path in the same ACT
    table set as exp (no sqrt-set loads between exp batches).

All LN/softmax stats and residuals fp32.
"""

import sys
import os

for _p in ("/opt/trn_rl_repo",):
    if _p not in sys.path and os.path.isdir(_p):
        sys.path.insert(0, _p)

import numpy as np
from contextlib import ExitStack

from concourse import bacc, bass, tile, mybir
from concourse.bass_utils import run_bass_kernel_spmd

F32 = mybir.dt.float32
F32R = mybir.dt.float32r
BF = mybir.dt.bfloat16
F8 = mybir.dt.float8e4
AF = mybir.ActivationFunctionType
OP = mybir.AluOpType
DR = mybir.MatmulPerfMode.DoubleRow
F8E5 = mybir.dt.float8e5

NCORES = 8
L, B, D, H = 2048, 4, 1024, 16
DK = D // H            # 64
DFF = 4 * D            # 4096
NOUT = L // 2          # 1024 tokens owned per core
P = 128
QB = 512               # qb block (half of NOUT)
NKT = L // P           # 16 kpos tiles
VW = DK + 1            # 65 v cols per head incl. ones col
EPS = 1e-5
SCALE = 1.0 / np.sqrt(DK)
DCH = D // P           # 8
MFF = DFF // P         # 32
WSC = 8.0              # host-side weight scale for fp8 qkvo
EXPB = -3.0            # softmax shift: exp(s + EXPB), keeps p < 240


def build_nc(debug=False):
    nc = bacc.Bacc("TRN2")
    dbg = {}
    if debug:
        dbg["kT"] = nc.declare_dram_parameter("d_kT", [P, DCH * L], F8,
                                              isOutput=True)
        dbg["qT"] = nc.declare_dram_parameter("d_qT", [P, DCH * NOUT], F8,
                                              isOutput=True)
        dbg["v"] = nc.declare_dram_parameter("d_v", [P, NKT * H * VW], F8,
                                             isOutput=True)
        dbg["p7"] = nc.declare_dram_parameter("d_p7", [P, 2 * NKT * QB], F8,
                                              isOutput=True)
        dbg["attnT"] = nc.declare_dram_parameter("d_attnT", [P, DCH * QB],
                                                 F8, isOutput=True)
        dbg["y1"] = nc.declare_dram_parameter("d_y1", [P, (NOUT // P) * D],
                                              BF, isOutput=True)
        dbg["h"] = nc.declare_dram_parameter("d_h", [P, MFF * QB], BF,
                                             isOutput=True)

    xt_e = nc.declare_dram_parameter("xt", [D, L], F8, isOutput=False)
    xr_e = nc.declare_dram_parameter("xr", [NOUT, D], BF, isOutput=False)
    # wqkvo: [D, 4D] fp8 = Wq|Wk|Wv|Wo (each x8)
    wqkvo_e = nc.declare_dram_parameter("wqkvo", [D, 4 * D], F8,
                                        isOutput=False)
    w1_e = nc.declare_dram_parameter("w1", [D, DFF], BF, isOutput=False)
    w2_e = nc.declare_dram_parameter("w2", [DFF, D], BF, isOutput=False)
    # brow: bq(D)|bk(D)|b1(DFF)|b2(D)|g1(D)|g2(D)|be2(D) fp32
    brow_e = nc.declare_dram_parameter("brow", [1, 6 * D + DFF], F32,
                                       isOutput=False)
    out_e = nc.declare_dram_parameter("out", [NOUT, D], BF, isOutput=True)
    wq_e = wqkvo_e[:, 0:D]
    wk_e = wqkvo_e[:, D:2 * D]
    wv_e = wqkvo_e[:, 2 * D:3 * D]
    wo_e = wqkvo_e[:, 3 * D:4 * D]
    bq_e = brow_e[0:1, 0:D]
    bk_e = brow_e[0:1, D:2 * D]
    b1_e = brow_e[0:1, 2 * D:2 * D + DFF]
    b2_e = brow_e[0:1, 2 * D + DFF:3 * D + DFF]
    g1_e = brow_e[0:1, 3 * D + DFF:4 * D + DFF]
    g2_e = brow_e[0:1, 4 * D + DFF:5 * D + DFF]
    be2_e = brow_e[0:1, 5 * D + DFF:6 * D + DFF]

    def r32(ap):
        return ap.bitcast(F32R)

    with tile.TileContext(nc) as tc, ExitStack() as ctx:
        persist = ctx.enter_context(tc.tile_pool(name="persist", bufs=1))

        # ---- constants ----
        ones_row = persist.tile([1, P], F32R)
        nc.vector.memset(ones_row[:].bitcast(F32), 1.0)
        eps_t = persist.tile([P, 1], F32)
        nc.vector.memset(eps_t[:], EPS)
        expb_t = persist.tile([P, 1], F32)
        nc.vector.memset(expb_t[:], EXPB)

        bq_pp = persist.tile([P, DCH], F32)
        nc.sync.dma_start(bq_pp[:], bq_e.rearrange("o (m p) -> (o p) m", p=P))
        bk_pp = persist.tile([P, DCH], F32)
        nc.sync.dma_start(bk_pp[:], bk_e.rearrange("o (m p) -> (o p) m", p=P))
        b1_pp = persist.tile([P, DFF // P], F32)
        nc.sync.dma_start(b1_pp[:], b1_e.rearrange("o (m p) -> (o p) m", p=P))

        def bcast_row(src_e, n, name, pool, row_pool, psum_pool):
            row = row_pool.tile([1, n], F32R, name=f"{name}_row", tag="row")
            nc.sync.dma_start(row[:], src_e[0:1, :].bitcast(F32R))
            bc = pool.tile([P, n], BF, name=f"{name}_bc")
            for j in range(n // QB):
                ps = psum_pool.tile([P, QB], F32, name=f"{name}_ps{j}", tag="bc_ps")
                nc.tensor.matmul(ps[:], r32(ones_row[:1, :]),
                                 r32(row[:1, j * QB:(j + 1) * QB]),
                                 start=True, stop=True)
                nc.scalar.copy(bc[:, j * QB:(j + 1) * QB], ps[:])
            return bc

        stBC = ExitStack()
        rowP = stBC.enter_context(tc.tile_pool(name="rowP", bufs=2))
        bcPs = stBC.enter_context(tc.tile_pool(name="bcPs", bufs=2, space="PSUM"))
        g1_bc = bcast_row(g1_e, D, "g1", persist, rowP, bcPs)
        b2_bc = bcast_row(b2_e, D, "b2", persist, rowP, bcPs)
        g2_bc = bcast_row(g2_e, D, "g2", persist, rowP, bcPs)
        be2_bc = bcast_row(be2_e, D, "be2", persist, rowP, bcPs)
        stBC.close()

        # ---- persistent activations ----
        big = ctx.enter_context(tc.tile_pool(name="big", bufs=1))
        kT_sb = big.tile([P, DCH, L], F8)     # 8*K^T: head h at part (h%2)*64
        qT_sb = big.tile([P, DCH, NOUT], F8)  # 8*Q^T, same packing
        v_sb = big.tile([P, NKT, H, VW], F8)  # 8*V natural + ones col
        y1bf = big.tile([P, NOUT // P, D], BF)  # LN1 out (residual for LN2)

        # ---- pools live through the whole pipeline ----
        poolB = ctx.enter_context(tc.tile_pool(name="poolB", bufs=1))
        poolC = ctx.enter_context(tc.tile_pool(name="poolC", bufs=2))
        wC = ctx.enter_context(tc.tile_pool(name="wC", bufs=1))
        w1p = ctx.enter_context(tc.tile_pool(name="w1p", bufs=2))
        w2p = ctx.enter_context(tc.tile_pool(name="w2p", bufs=2))
        psS = ctx.enter_context(tc.tile_pool(name="psS", bufs=2, space="PSUM"))
        psAcc = ctx.enter_context(tc.tile_pool(name="psAcc", bufs=2, space="PSUM"))
        psF2 = ctx.enter_context(tc.tile_pool(name="psF2", bufs=1, space="PSUM"))

        def pf2(i):
            """ping-pong [P,512] PSUM tiles from the psF2 pool's two banks"""
            return psF2.tile([P, QB], F32, name=f"pf{i % 2}", tag=f"pf{i % 2}")

        attnT = poolB.tile([P, DCH, QB], F8, name="attnT", tag="attnT", bufs=1)
        y1T = poolB.tile([P, DCH, QB], BF, name="y1T", tag="y1T", bufs=1)
        h_sb = poolB.tile([P, MFF, QB], BF, name="h_sb", tag="h_sb", bufs=1)

        w1_v = w1_e.rearrange("(c p) (m q) -> p c m q", p=P, q=P)
        w2_v = w2_e.rearrange("(cc p) n -> p cc n", p=P)
        wk_v = wk_e.rearrange("(c p) (m q) -> p c m q", p=P, q=P)
        wq_v = wq_e.rearrange("(c p) (m q) -> p c m q", p=P, q=P)

        # =================== phase A: xT, V-proj, seed K/Q ===================
        stA = ExitStack()
        poolX = stA.enter_context(tc.tile_pool(name="poolX", bufs=1))
        wA = stA.enter_context(tc.tile_pool(name="wA", bufs=2))

        xT = poolX.tile([P, DCH, L], F8, name="xT", tag="xbuf")
        nc.sync.dma_start(xT[:], xt_e.rearrange("(c p) n -> p c n", p=P))

        wv_v = wv_e.rearrange("(c p) n -> p c n", p=P)
        wv_cur = {}

        def wv_load(qtr):
            wv_sb = wA.tile([P, DCH, QB // 2], F8, name=f"wv{qtr}", tag="wv",
                            bufs=1)
            nc.sync.dma_start(wv_sb[:],
                              wv_v[:, :, qtr * (QB // 2):(qtr + 1) * (QB // 2)])
            wv_cur[qtr] = wv_sb

        def v_pieces(qtr):
            """quarter qtr covers heads 4*qtr..4*qtr+3 (256 channels)"""
            pieces = []

            def mk(kt):
                def go():
                    ps = pf2(kt)
                    for c2 in range(4):
                        nc.tensor.matmul(ps[:, 0:QB // 2],
                                         xT[:, 2 * c2:2 * c2 + 2,
                                            kt * P:(kt + 1) * P],
                                         wv_cur[qtr][:, 2 * c2:2 * c2 + 2, :],
                                         start=(c2 == 0), stop=(c2 == 3),
                                         perf_mode=DR)
                    nc.vector.tensor_copy(
                        v_sb[:, kt, qtr * (H // 4):(qtr + 1) * (H // 4), 0:DK],
                        ps[:, 0:QB // 2].rearrange("p (hh e) -> p hh e", e=DK))
                return go

            for kt in range(NKT):
                pieces.append(mk(kt))
            return pieces

        wv_load(0)
        for kt in range(NKT):
            nc.vector.memset(v_sb[:, kt, :, DK:DK + 1], 1.0)

        # K/Q projection for one head-pair chunk m, as interleavable pieces.
        kq_w = {}

        def kq_load(mc, which):
            w_m = wA.tile([P, DCH, 4, P], F8, name=f"w{which}{mc}",
                          tag=f"w{which}", bufs=1)
            src = wk_v if which == "k" else wq_v
            nc.sync.dma_start(w_m[:], src[:, :, mc * 4:(mc + 1) * 4, :])
            kq_w[(which, mc)] = w_m

        def kq_pieces(m):
            """returns a list of closures: K-proj (4) + Q-proj (2) for chunk m"""
            pieces = []
            mc, mi = m // 4, m % 4

            def mk_k(tq):
                def go():
                    w_m = kq_w[("k", mc)]
                    ps = pf2(tq)
                    for c2 in range(4):
                        nc.tensor.matmul(ps[:],
                                         w_m[:, 2 * c2:2 * c2 + 2, mi, :],
                                         xT[:, 2 * c2:2 * c2 + 2,
                                            tq * QB:(tq + 1) * QB],
                                         start=(c2 == 0), stop=(c2 == 3),
                                         perf_mode=DR)
                    nc.vector.tensor_scalar(
                        kT_sb[:, m, tq * QB:(tq + 1) * QB], ps[:],
                        bk_pp[:, m:m + 1], None, op0=OP.add)
                return go

            def mk_q(tq):
                def go():
                    w_m = kq_w[("q", mc)]
                    ps = pf2(tq)
                    for c2 in range(4):
                        nc.tensor.matmul(ps[:],
                                         w_m[:, 2 * c2:2 * c2 + 2, mi, :],
                                         xT[:, 2 * c2:2 * c2 + 2,
                                            tq * QB:(tq + 1) * QB],
                                         start=(c2 == 0), stop=(c2 == 3),
                                         perf_mode=DR)
                    nc.vector.tensor_scalar(
                        qT_sb[:, m, tq * QB:(tq + 1) * QB], ps[:],
                        bq_pp[:, m:m + 1], None, op0=OP.add)
                return go

            for tq in range(4):
                pieces.append(mk_k(tq))
            for tq in range(2):
                pieces.append(mk_q(tq))
            return pieces

        kq_load(0, "k")
        kq_load(0, "q")
        for piece in kq_pieces(0):  # seed chunk 0 so B0's first pair is ready
            piece()

        # =================== attention (exp-bound, PE filler) ===============
        def attention(qb, fillers_by_pair):
            """scores + softmax + attn@V for qb's 512 tokens, 8 head pairs.
            fillers_by_pair[m]: closures issued during pair m's exp-gated
            segments (all complete before pair m+1 begins)."""
            qsl = slice(qb * QB, (qb + 1) * QB)

            for m in range(DCH):
                pieces = list(fillers_by_pair[m]) if m < len(fillers_by_pair) \
                    else []
                np_ = len(pieces)

                def fill(slot):
                    lo = (np_ * slot) // 8
                    hi = (np_ * (slot + 1)) // 8 if slot < 8 else np_
                    for i in range(min(lo, np_), min(hi, np_)):
                        pieces[i]()
                p_pair = poolB.tile([P, 2, NKT, QB], F8E5, name="p_pair",
                                    tag="p_pair", bufs=1)
                for kt in range(NKT):
                    ps2 = psS.tile([P, 2, QB], F32, name="s_ps", tag="s_ps")
                    for par in range(2):
                        b0 = par * DK
                        nc.tensor.matmul(ps2[:, par, :],
                                         kT_sb[b0:b0 + DK, m,
                                               kt * P:(kt + 1) * P],
                                         qT_sb[b0:b0 + DK, m, qsl],
                                         start=True, stop=True)
                    nc.scalar.activation(p_pair[:, :, kt, :],
                                         ps2[:], AF.Exp,
                                         scale=float(SCALE / (WSC * WSC)),
                                         bias=expb_t[:])
                    if kt % 2 == 1:
                        fill(kt // 2)
                for par in range(2):
                    h = 2 * m + par
                    b0 = par * DK
                    acc = psAcc.tile([VW, QB], F32, name="acc65", tag="acc65")
                    for g in range(NKT // 2):
                        nc.tensor.matmul(acc[:],
                                         v_sb[:, 2 * g:2 * g + 2, h, :],
                                         p_pair[:, par, 2 * g:2 * g + 2, :],
                                         start=(g == 0), stop=(g == 7),
                                         perf_mode=DR)
                    rec = poolB.tile([1, QB], BF, name="rec", tag="rec")
                    with nc.allow_low_precision(
                            reason="softmax denom reciprocal in bf16; "
                                   "attn path error diluted ~30x by residual"):
                        nc.vector.reciprocal(rec[:], acc[DK:DK + 1, :])
                    bcr = poolB.tile([DK, QB], BF, name="bcr", tag="bcr")
                    nc.gpsimd.partition_broadcast(bcr[:], rec[:])
                    nc.vector.tensor_tensor(attnT[b0:b0 + DK, m, :],
                                            acc[0:DK, :], bcr[:], op=OP.mult)
                fill(8)

        # =================== O-proj + LN1 for one qb ========================
        c_state = {}

        def oproj_stats(qb):
            """O-proj (fp8 DR) + residual (bf16) + LN1 stats. DVE work here
            overlaps a following PE-bound phase (ffn2 of the other qb)."""
            ress, aggrs = [], []
            for tb in range(QB // P):
                tt = qb * (QB // P) + tb
                xr_t = poolC.tile([P, D], BF, name="xr_t", tag="xr_t", bufs=2)
                nc.sync.dma_start(xr_t[:], xr_e[tt * P:(tt + 1) * P, :])
                res = poolC.tile([P, D], BF, name=f"res1_{tb}",
                                 tag=f"resC_{tb}", bufs=1)
                for hf in range(2):
                    sl = slice(hf * QB, (hf + 1) * QB)
                    ps = pf2(2 * tb + hf)
                    for c2 in range(4):
                        nc.tensor.matmul(
                            ps[:], attnT[:, 2 * c2:2 * c2 + 2,
                                         tb * P:(tb + 1) * P],
                            wo_sb[:, 2 * c2:2 * c2 + 2, sl],
                            start=(c2 == 0), stop=(c2 == 3), perf_mode=DR)
                    nc.vector.scalar_tensor_tensor(
                        res[:, sl], ps[:], 1.0 / (WSC * WSC), xr_t[:, sl],
                        op0=OP.mult, op1=OP.add)
                stats = poolC.tile([P, 2, 6], F32, name="stats1", tag="stats")
                for c_ in range(2):
                    nc.vector.bn_stats(stats[:, c_, :],
                                       res[:, c_ * QB:(c_ + 1) * QB])
                if tb == 0:
                    aggr4 = poolC.tile([P, 4, 2], F32, name="aggr1",
                                       tag="aggr1", bufs=1)
                nc.vector.bn_aggr(aggr4[:, tb, :], stats[:])
                ress.append(res)
            c_state[qb] = (ress, aggr4)

        def ln1_apply(qb):
            ress, aggr4 = c_state.pop(qb)
            std4 = poolC.tile([P, 4], F32, name="std1", tag="std1")
            nc.scalar.activation(std4[:], aggr4[:, :, 1], AF.Sqrt,
                                 bias=eps_t[:])
            rstd4 = poolC.tile([P, 4], F32, name="rstd1", tag="rstd1")
            nc.vector.reciprocal(rstd4[:], std4[:])
            for tb in range(QB // P):
                tt = qb * (QB // P) + tb
                ytn = poolC.tile([P, D], BF, name="ytn", tag="ytn", bufs=2)
                nc.vector.tensor_scalar(ytn[:], ress[tb][:],
                                        aggr4[:, tb, 0:1],
                                        rstd4[:, tb:tb + 1],
                                        op0=OP.subtract, op1=OP.mult)
                nc.vector.tensor_tensor(y1bf[:, tt, :], ytn[:], g1_bc[:],
                                        op=OP.mult)
                nc.sync.dma_start_transpose(y1T[:, :, tb * P:(tb + 1) * P],
                                            ytn[:])

        def oproj_ln1(qb):
            oproj_stats(qb)
            ln1_apply(qb)

        # =================== FFN1 pieces (interleaved into B1) ==============
        def ffn1_pieces(qb):
            """z = y1 @ W1 + b1 -> h_sb (raw, bf16); gelu applied later."""
            pieces = []

            def mk_load(mc):
                def go():
                    w1_blk = w1p.tile([P, DCH, 2, P], BF, name="w1_blk",
                                      tag="w1")
                    kq_w[("w1", mc)] = w1_blk
                    nc.sync.dma_start(w1_blk[:],
                                      w1_v[:, :, mc * 2:(mc + 1) * 2, :])
                return go

            def mk_chunk(mc, mi):
                def go():
                    m = mc * 2 + mi
                    w1_blk = kq_w[("w1", mc)]
                    ps = pf2(m)
                    for k in range(DCH):
                        nc.tensor.matmul(ps[:], w1_blk[:, k, mi, :],
                                         y1T[:, k, :],
                                         start=(k == 0), stop=(k == DCH - 1))
                    nc.vector.tensor_scalar(h_sb[:, m, :], ps[:],
                                            b1_pp[:, m:m + 1], None,
                                            op0=OP.add)
                return go

            for mc in range(MFF // 2):
                pieces.append(mk_load(mc))
                for mi in range(2):
                    pieces.append(mk_chunk(mc, mi))
            return pieces

        def gelu_all():
            for gg in range(2):
                nc.scalar.activation(h_sb[:, gg * 16:(gg + 1) * 16, :],
                                     h_sb[:, gg * 16:(gg + 1) * 16, :], AF.Gelu)

        # =================== FFN2 + LN2 + out for one qb ====================
        def ffn2(qb):
            res2s = [poolC.tile([P, D], BF, name=f"res2_{j}", tag=f"resw_{j}",
                                bufs=1) for j in range(4)]
            for hf in range(2):
                sl = slice(hf * QB, (hf + 1) * QB)
                # 4 accumulators, one full PSUM bank each, borrowed from the
                # (idle during FFN2) psS pool
                acc_ab = psS.tile([P, 2, QB], F32, name="f_ab", tag="s_ps")
                acc_cd = psS.tile([P, 2, QB], F32, name="f_cd", tag="s_ps")
                accs = [acc_ab[:, 0, :], acc_ab[:, 1, :],
                        acc_cd[:, 0, :], acc_cd[:, 1, :]]
                for cc in range(8):
                    w2_c = w2p.tile([P, DCH // 2, QB], BF, name="w2_c",
                                    tag="w2", bufs=2)
                    nc.sync.dma_start(
                        w2_c[:], w2_v[:, cc * 4:(cc + 1) * 4, sl])
                    for ci in range(DCH // 2):
                        c = cc * 4 + ci
                        for j in range(4):
                            nc.tensor.matmul(
                                accs[j], h_sb[:, c, j * P:(j + 1) * P],
                                w2_c[:, ci, :],
                                start=(c == 0), stop=(c == MFF - 1))
                for j in range(4):
                    tb = qb * 4 + j
                    nc.vector.tensor_tensor(res2s[j][:, sl], accs[j],
                                            b2_bc[:, sl], op=OP.add)
                    nc.vector.tensor_tensor(res2s[j][:, sl], res2s[j][:, sl],
                                            y1bf[:, tb, sl], op=OP.add)
            agg4 = poolC.tile([P, 4, 2], F32, name="aggr2", tag="aggr2",
                              bufs=1)
            for j in range(4):
                res2 = res2s[j]
                stats = poolC.tile([P, 2, 6], F32, name="stats2", tag="stats2")
                for c_ in range(2):
                    nc.vector.bn_stats(stats[:, c_, :],
                                       res2[:, c_ * QB:(c_ + 1) * QB])
                nc.vector.bn_aggr(agg4[:, j, :], stats[:])
            std4 = poolC.tile([P, 4], F32, name="std2", tag="std2")
            nc.scalar.activation(std4[:], agg4[:, :, 1], AF.Sqrt,
                                 bias=eps_t[:])
            rstd4 = poolC.tile([P, 4], F32, name="rstd2", tag="rstd2")
            nc.vector.reciprocal(rstd4[:], std4[:])
            for j in range(4):
                tb = qb * 4 + j
                o_t = poolC.tile([P, D], BF, name="oo_t", tag="yt", bufs=2)
                nc.vector.tensor_scalar(o_t[:], res2s[j][:], agg4[:, j, 0:1],
                                        rstd4[:, j:j + 1],
                                        op0=OP.subtract, op1=OP.mult)
                nc.vector.tensor_tensor(o_t[:], o_t[:], g2_bc[:], op=OP.mult)
                o_bf = poolC.tile([P, D], BF, name="o_bf", tag="ytn", bufs=2)
                nc.vector.tensor_tensor(o_bf[:], o_t[:], be2_bc[:], op=OP.add)
                nc.sync.dma_start(out_e[tb * P:(tb + 1) * P, :], o_bf[:])

        # ========================= the pipeline =============================
        wo_sb = wC.tile([P, DCH, D], F8)
        nc.sync.dma_start(wo_sb[:], wo_e.rearrange("(c p) n -> p c n", p=P))

        # B0: attention(qb0) absorbing K/Q projection of chunks 1..7.
        # During pair m we issue chunk m+1's pieces, so they are complete
        # (program-order) before pair m+1's scores read them.
        kq_load(1, "k")
        kq_load(1, "q")
        b0_fillers = [kq_pieces(m + 1) for m in range(DCH - 1)] + [[]]
        b0_fillers[0] = v_pieces(0) + b0_fillers[0]
        b0_fillers[1] = [lambda: wv_load(1)] + v_pieces(1) + b0_fillers[1]
        b0_fillers[2] = [lambda: wv_load(2)] + v_pieces(2) + b0_fillers[2]
        b0_fillers[3] = [lambda: wv_load(3)] + v_pieces(3) + b0_fillers[3]

        attention(0, b0_fillers)
        stA.close()          # xT / wA no longer needed
        if debug:
            nc.sync.dma_start(dbg["kT"][:, :], kT_sb[:].rearrange("p a b -> p (a b)"))
            nc.sync.dma_start(dbg["qT"][:, :], qT_sb[:].rearrange("p a b -> p (a b)"))
            nc.sync.dma_start(dbg["v"][:, :], v_sb[:].rearrange("p a b c -> p (a b c)"))
            nc.sync.dma_start(dbg["attnT"][:, :], attnT[:].rearrange("p a b -> p (a b)"))
        oproj_ln1(0)

        # B1: attention(qb1) absorbing FFN1(qb0)
        f1 = ffn1_pieces(0)
        b1_fillers = [f1[(len(f1) * m) // DCH:(len(f1) * (m + 1)) // DCH]
                      for m in range(DCH)]
        attention(1, b1_fillers)
        if debug:
            nc.sync.dma_start(dbg["y1"][:, :], y1bf[:].rearrange("p a b -> p (a b)"))
        gelu_all()
        if debug:
            nc.sync.dma_start(dbg["h"][:, :], h_sb[:].rearrange("p a b -> p (a b)"))
        oproj_stats(1)
        ln1_apply(1)
        ffn2(0)

        # FFN(qb1): nothing left to hide under
        for piece in ffn1_pieces(1):
            piece()
        gelu_all()
        ffn2(1)

    nc.finalize()
    return nc


NP_BF16 = mybir.dt.np(BF)
NP_F8 = mybir.dt.np(F8)


def make_in_maps(inputs):
    x = np.ascontiguousarray(np.asarray(inputs["x"], dtype=np.float32))
    w = {k: np.asarray(v, dtype=np.float32) for k, v in inputs.items() if k != "x"}
    row = lambda a: np.ascontiguousarray(a.reshape(1, -1).astype(np.float32))
    bf = lambda a: np.ascontiguousarray(a.astype(NP_BF16))
    f8 = lambda a: np.ascontiguousarray(
        np.clip(a, -240.0, 240.0).astype(NP_F8))

    # residual offset: bo + bv@Wo (softmax rows sum to 1)
    resoff = (w["bo"] + w["bv"] @ w["Wo"]).reshape(1, D).astype(np.float32)

    wqkvo = np.concatenate([f8(w["Wq"] * WSC), f8(w["Wk"] * WSC),
                            f8(w["Wv"] * WSC), f8(w["Wo"] * WSC)], axis=1)
    brow = np.concatenate([
        row(w["bq"] * WSC), row(w["bk"] * WSC),
        row(w["b1"] + w["be1"] @ w["W1"]), row(w["b2"] + w["be1"]),
        row(w["g1"]), row(w["g2"]), row(w["be2"])], axis=1)
    shared = dict(
        wqkvo=np.ascontiguousarray(wqkvo),
        w1=bf(w["g1"][:, None] * w["W1"]), w2=bf(w["W2"]),
        brow=np.ascontiguousarray(brow),
    )
    in_maps = []
    for i in range(NCORES):
        g, hf = i // 2, i % 2
        xb = x[:, g, :]                          # [L, D]
        # roll so this core's 1024 tokens come first (kpos order is
        # irrelevant to attention as long as K and V share it)
        xroll = np.roll(xb, -hf * NOUT, axis=0)
        m = dict(shared)
        m["xt"] = f8(xroll.T)
        m["xr"] = bf(xroll[:NOUT] + resoff)
        in_maps.append(m)
    return in_maps


def assemble(results):
    full = np.empty((L, B, D), np.float32)
    for i in range(NCORES):
        g, hf = i // 2, i % 2
        full[hf * NOUT:(hf + 1) * NOUT, g, :] = \
            np.asarray(results[i]["out"]).astype(np.float32)
    return full


_NC_CACHE = None


def _get_nc():
    global _NC_CACHE
    if _NC_CACHE is None:
        _NC_CACHE = build_nc()
    return _NC_CACHE


def kernel(**inputs):
    nc = _get_nc()
    in_maps = make_in_maps(inputs)
    res = run_bass_kernel_spmd(nc, in_maps, list(range(NCORES)))
    return assemble(res.results)


if __name__ == "__main__":
    nc = build_nc()
    print("built ok; instructions:", len(nc.inst_map))
